# revision 19
# baseline (speedup 1.0000x reference)
"""BranchLayer kernel for 8 Trainium2 NeuronCores.

Math: out[b, c] = sum_k x[b, idx[k, c]] * w[k, c], with last-write-wins on
duplicate (idx[k,c], c) pairs — i.e. out = x @ dense where
dense[i, c] = w[k_last, c] for the last k with idx[k, c] == i.

Strategy (sharding_hint): shard the COLS=16384 column dim of dense across the
8 cores (2048 columns each); x is replicated. The host scatters w into dense
(cheap index bookkeeping), converts to bf16, and each core runs a pipelined
TensorE matmul x[128, 4096] @ dense_shard[4096, 2048] with fp32 PSUM
accumulation, overlapping the dense DMA-in with the matmuls.
"""

import numpy as np
import ml_dtypes

import concourse.bass as bass
import concourse.bacc as bacc
import concourse.mybir as mybir
import concourse.tile as tile
from concourse import bass_utils

F16 = np.float16

# Problem shape (hardcoded per task contract).
N_IN = 4096
N_NPB = 64
N_B = 64
N_NEXT_H = 256
COLS = N_B * N_NEXT_H  # 16384
BATCH = 128
N_CORES = 8

COLS_PER_CORE = COLS // N_CORES  # 2048
N_BLOCK = 512                    # output columns per PSUM block (one bank)
NUM_BLOCKS = COLS_PER_CORE // N_BLOCK  # 4
N_ITILES = N_IN // 128           # 32 contraction tiles

_CACHE = {}


def _build_program(repeats=1, dbufs=4, chunks=8, warmup=8, out_eng="gpsimd"):
    """One SPMD Bass program; all 8 cores run it on different dense shards.

    repeats>1 loops the whole pipeline inside one NEFF — used only for
    repeat-delta HW timing in test.py (tunnel overhead cancels).
    dbufs: dense-tile pool slots (4 = every block's DMA in flight at start).
    chunks: dense DMA chunks per block (finer ⇒ earlier first matmul and a
    shorter post-DMA tail on the last block).
    warmup: dummy N=512 matmuls issued at t=0 against the (garbage) dense
    tile to flip the PE HAM throttle to full clock during the DMA fill.
    """
    key = ("nc", repeats, dbufs, chunks, warmup, out_eng)
    if key in _CACHE:
        return _CACHE[key]

    nc = bacc.Bacc(
        "TRN2",
        target_bir_lowering=False,
        debug=False,
        enable_asserts=False,
        num_devices=N_CORES,
    )
    # xT[il, t*128 + b] = x[b, t*128 + il]  (lhsT tiles, bf16)
    xT = nc.dram_tensor("xT", [128, N_IN], mybir.dt.float16, kind="ExternalInput").ap()
    # dns[n, il, t*N_BLOCK + c'] = dense[t*128 + il, n*N_BLOCK + c'] (per-core shard)
    dns = nc.dram_tensor(
        "dns", [NUM_BLOCKS, 128, N_ITILES * N_BLOCK], mybir.dt.float16,
        kind="ExternalInput",
    ).ap()
    out = nc.dram_tensor(
        "out", [BATCH, COLS_PER_CORE], mybir.dt.float32, kind="ExternalOutput"
    ).ap()

    with tile.TileContext(nc) as tc:
        with (
            tc.tile_pool(name="xp", bufs=1) as xp,
            tc.tile_pool(name="dp", bufs=dbufs) as dp,
            tc.tile_pool(name="op", bufs=2) as op,
            tc.tile_pool(name="pp", bufs=2, space="PSUM") as pp,
        ):
            # xT rides the scalar queue so the first dense chunk (sync
            # queue) starts streaming immediately at kernel start.
            x_sb = xp.tile([128, N_IN], mybir.dt.float16)
            nc.scalar.dma_start(out=x_sb[:], in_=xT[:])

            # Dense loads: chunks alternating the two HWDGE queues (sync=SP,
            # scalar=ACT) — matmuls on chunk h start while chunk h+1 streams.
            # Out-DMAs ride the scalar queue so the sync queue never stalls
            # behind a compute-dependent wait.
            csize = N_ITILES * N_BLOCK // chunks
            t_per_chunk = N_ITILES // chunks
            qs = [nc.sync, nc.scalar]
            qi = 0
            warm_ps = pp.tile([BATCH, N_BLOCK], mybir.dt.float32, tag="warm")
            # PE warmup: dummy matmuls on x_sb (pure reads, no hazards with
            # the real matmuls) keep the PE busy during the DMA fill so the
            # HAM clock gate opens before the real work lands. Results go to
            # a scratch PSUM tile nobody reads.
            for _wmm in range(warmup):
                nc.tensor.matmul(
                    warm_ps[:],
                    x_sb[:, :128],
                    x_sb[:, :N_BLOCK],
                    start=True,
                    stop=True,
                )
            for _rep in range(repeats):
                for n in range(NUM_BLOCKS):
                    d_sb = dp.tile([128, N_ITILES * N_BLOCK], mybir.dt.float16)
                    for h in range(chunks):
                        qs[qi % 2].dma_start(
                            out=d_sb[:, h * csize:(h + 1) * csize],
                            in_=dns[n, :, h * csize:(h + 1) * csize],
                        )
                        qi += 1

                    ps = pp.tile([BATCH, N_BLOCK], mybir.dt.float32)
                    for t in range(N_ITILES):
                        nc.tensor.matmul(
                            ps[:],
                            x_sb[:, t * 128:(t + 1) * 128],
                            d_sb[:, t * N_BLOCK:(t + 1) * N_BLOCK],
                            start=(t == 0),
                            stop=(t == N_ITILES - 1),
                        )
                    o_sb = op.tile([BATCH, N_BLOCK], mybir.dt.float32)
                    nc.vector.tensor_copy(out=o_sb[:], in_=ps[:])
                    getattr(nc, out_eng).dma_start(
                        out=out[:, n * N_BLOCK:(n + 1) * N_BLOCK], in_=o_sb[:]
                    )

    nc.compile()
    aps = {"xT": xT, "dns": dns, "out": out}
    _CACHE[key] = (nc, aps)
    return nc, aps


def _prepare_inputs(x, w, idx):
    x = np.asarray(x, dtype=np.float32)
    w = np.asarray(w, dtype=np.float32)
    idx = np.asarray(idx)

    # Scatter with last-write-wins (ascending k ⇒ later k overwrites earlier,
    # matching torch's index_put / the reference's keep-mask + scatter-add).
    dense = np.zeros((N_IN, COLS), dtype=np.float32)
    cols = np.arange(COLS)
    for k in range(N_NPB):
        dense[idx[k], cols] = w[k]

    # lhsT layout: xT[il, t, b] = x[b, t*128 + il]
    xT = np.ascontiguousarray(
        x.T.reshape(N_ITILES, 128, BATCH).transpose(1, 0, 2).reshape(128, N_IN)
    ).astype(F16)

    in_maps = []
    for core in range(N_CORES):
        dc = dense[:, core * COLS_PER_CORE:(core + 1) * COLS_PER_CORE]
        # D[n, il, t, c'] = dc[t*128 + il, n*N_BLOCK + c']
        D = np.ascontiguousarray(
            dc.reshape(N_ITILES, 128, NUM_BLOCKS, N_BLOCK)
            .transpose(2, 1, 0, 3)
            .reshape(NUM_BLOCKS, 128, N_ITILES * N_BLOCK)
        ).astype(F16)
        in_maps.append({"xT": xT, "dns": D})
    return in_maps


def _run(in_maps, trace=False):
    nc, _ = _build_program()
    res = bass_utils.run_bass_kernel_spmd(
        nc, in_maps, core_ids=list(range(N_CORES)), trace=trace
    )
    _CACHE["last_results"] = res
    return res


def kernel(x, w, idx):
    in_maps = _prepare_inputs(x, w, idx)
    try:
        res = _run(in_maps, trace=False)
    except Exception:
        # A previously wedged device can fail the first attach; one retry
        # on a fresh execution is usually enough (device resets on attach).
        import time
        time.sleep(2.0)
        res = _run(in_maps, trace=False)
    out = np.concatenate([r["out"] for r in res.results], axis=1)
    return out.reshape(BATCH, N_B, N_NEXT_H).astype(np.float32)
